# revision 3
# baseline (speedup 1.0000x reference)
"""Trainium2 Bass kernel for per-frame multi-head attention with partial RoPE.

Problem (hardcoded): b=2, N=4096, dim=512, H=8, DH=64, f=4 frames of n=1024
tokens, ROT_DIM=32 partial rotary, softmax attention per (b, h, frame) block,
then output projection.

Sharding: 8 cores = (batch, frame) pairs. Each core runs all 8 heads for one
1024-token frame — fully independent, no collectives.

v2 design notes (per core):
  - The ACT (scalar) engine is the hard floor: 64 exps of [128,1024] at
    ~1.04us each (~66us). It does ONLY exp; the table set is preloaded by a
    junk exp at t=0. All PSUM->SBUF evacuation rides DVE (plus two ACT copies
    in the prologue while ACT is otherwise idle).
  - PSUM (8 banks): tag "ps" 2x[128,1024] is the S^T ring (also carries
    deferred QKV chunks, V chunks, and the output projection); tag "po"
    2x[128,1024] holds the two PV accumulators of the current head pair.
  - Softmax denominator for free: the PV stationary is widened to 128 cols
    where cols 64..127 are all-ones, so po rows 64..127 hold l[i] broadcast
    across 64 partitions. Normalize = reciprocal_approx_fast + one mul.
  - S^T runs as concurrent PE row-tiles (two heads at tile_position (0,0) /
    (64,0)); q-scale is folded into W_qkv on the host; exp is applied without
    max-subtraction (logits ~N(0,1)).
  - RoPE uses host-precomputed masked cos/sin tiles; deferred chunks push the
    second mul + add to the (otherwise idle) GpSimd engine.
  - ~30 junk matmuls at t=0 warm the PE HAM clock gate during the DMA fill;
    DMA order puts xT + the first q/k weight stripes first.
  - Output is written as fp16 [512,1024]; the host transposes/casts.
"""

from contextlib import ExitStack

import numpy as np

import concourse.bass as bass
import concourse.tile as tile
from concourse import bacc
from concourse import mybir
from concourse.bass_utils import run_bass_kernel_spmd

F32 = mybir.dt.float32
F16 = mybir.dt.float16
BF16 = mybir.dt.bfloat16

B, N, DIM = 2, 4096, 512
H, DH = 8, 64
NF = 4                # frames
NTOK = 1024           # tokens per frame
ROT = 32
SCALE = DH ** -0.5
NCORES = 8

PAIRSWAP = [i ^ 1 for i in range(32)]
MM_DT = F16
N_WARM = 30


def build_program():
    """Build the single-core Bass/Tile program (SPMD across 8 cores)."""
    nc = bacc.Bacc(trn_type="TRN2", target_bir_lowering=False, debug=False)

    xt_d = nc.dram_tensor("xt", [DIM, NTOK], MM_DT, kind="ExternalInput").ap()
    wqkv_d = nc.dram_tensor("wqkv", [DIM, 3 * H * DH], MM_DT, kind="ExternalInput").ap()
    wout_d = nc.dram_tensor("wout", [H * DH, DIM], MM_DT, kind="ExternalInput").ap()
    bout_d = nc.dram_tensor("bout", [DIM], F32, kind="ExternalInput").ap()
    cosm_d = nc.dram_tensor("cosm", [128, NTOK], MM_DT, kind="ExternalInput").ap()
    sinm_d = nc.dram_tensor("sinm", [128, NTOK], MM_DT, kind="ExternalInput").ap()
    out_d = nc.dram_tensor("out_t", [DIM, NTOK], MM_DT, kind="ExternalOutput").ap()

    EXP = mybir.ActivationFunctionType.Exp

    with tile.TileContext(nc) as tc, ExitStack() as ctx:
        const = ctx.enter_context(tc.tile_pool(name="const", bufs=1))
        big = ctx.enter_context(tc.tile_pool(name="big", bufs=1))
        stage = ctx.enter_context(tc.tile_pool(name="stage", bufs=2))
        epool = ctx.enter_context(tc.tile_pool(name="E", bufs=8))
        rtp = ctx.enter_context(tc.tile_pool(name="rtp", bufs=2))
        psum = ctx.enter_context(tc.tile_pool(name="ps", bufs=2, space="PSUM"))

        # ---- SBUF tiles ----
        wqkv = const.tile([128, 4, 3 * H * DH], MM_DT, tag="wqkv", name="wqkv_sb")
        xT = const.tile([128, 4, NTOK], MM_DT, tag="xT", name="xT")
        wout = const.tile([128, 4, DIM], MM_DT, tag="wout", name="wout_sb")
        bout = const.tile([128, 4], F32, tag="bout", name="bout_sb")
        cosm = const.tile([128, NTOK], MM_DT, tag="cosm", name="cosm_sb")
        sinm = const.tile([128, NTOK], MM_DT, tag="sinm", name="sinm_sb")
        junkW = const.tile([128, 128], MM_DT, tag="junkW", name="junkW")
        junkX = const.tile([128, 128], MM_DT, tag="junkX", name="junkX")
        junkE = const.tile([128, 16], MM_DT, tag="junkE", name="junkE")

        # q chunks 0..3 then k chunks 0..3 (roped, fp16, feature-major)
        qk = [big.tile([128, NTOK], MM_DT, tag=f"qk{i}", name=f"qk{i}")
              for i in range(8)]
        # V token-major per j-chunk: [128 tok, head, DH cols | 64 ones cols]
        vsb = [big.tile([128, H, 128], MM_DT, tag=f"v{t}", name=f"v{t}")
               for t in range(8)]
        obar = [big.tile([128, NTOK], MM_DT, tag=f"ob{c}", name=f"ob{c}")
                for c in range(4)]
        outsb = [big.tile([128, NTOK], MM_DT, tag=f"os{c}", name=f"os{c}")
                 for c in range(4)]

        # ---- warmup: ACT table load + PE HAM while DMAs fill ----
        nc.vector.memset(junkW[:], 0.03125)
        nc.vector.memset(junkX[:], 0.03125)
        nc.scalar.activation(junkE[:], junkX[:, 0:16], EXP)
        for i in range(N_WARM):
            wps = psum.tile([128, 512], F32, tag="po", name="warm")
            nc.tensor.matmul(wps[:, 0:128], junkW[:], junkX[:],
                             start=True, stop=True)
        # ones columns of the PV stationaries (cols DH..127 stay 1.0)
        for t in range(8):
            nc.gpsimd.memset(vsb[t][:], 1.0)

        # ---- DMAs, critical-path order ----
        xt_r = xt_d.rearrange("(kc p) t -> p kc t", p=128)
        wqkv_r = wqkv_d.rearrange("(kc p) c -> p kc c", p=128)
        for tq in range(4):
            nc.sync.dma_start(xT[:, :, tq * 256:(tq + 1) * 256],
                              xt_r[:, :, tq * 256:(tq + 1) * 256])
        nc.sync.dma_start(wqkv[:, :, 0:128], wqkv_r[:, :, 0:128])          # q0
        nc.sync.dma_start(wqkv[:, :, 512:640], wqkv_r[:, :, 512:640])      # k0
        nc.sync.dma_start(wqkv[:, :, 1024:1536], wqkv_r[:, :, 1024:1536])  # v
        nc.sync.dma_start(cosm[:], cosm_d)
        nc.sync.dma_start(sinm[:], sinm_d)
        nc.sync.dma_start(wqkv[:, :, 128:512], wqkv_r[:, :, 128:512])      # q1-3
        nc.sync.dma_start(wqkv[:, :, 640:1024], wqkv_r[:, :, 640:1024])    # k1-3
        nc.sync.dma_start(wout[:], wout_d.rearrange("(kc p) c -> p kc c", p=128))
        nc.sync.dma_start(bout[:], bout_d.rearrange("(c p) -> p c", p=128))

        def qkv_chunk(idx, on_act):
            """Project one q/k chunk (2 heads) and rope it into qk[idx]."""
            col0 = idx * 128 if idx < 4 else 512 + (idx - 4) * 128
            pq = psum.tile([128, NTOK], F32, tag="ps", name="pq")
            for kc in range(4):
                for ih in range(2):
                    nc.tensor.matmul(
                        pq[:, ih * 512:(ih + 1) * 512],
                        wqkv[:, kc, col0:col0 + 128],
                        xT[:, kc, ih * 512:(ih + 1) * 512],
                        start=(kc == 0), stop=(kc == 3),
                    )
            cop = stage.tile([128, NTOK], MM_DT, tag="cop", name="cop")
            if on_act:
                nc.scalar.copy(cop[:], pq[:])
            else:
                nc.vector.tensor_copy(cop[:], pq[:])
            t1 = stage.tile([128, NTOK], MM_DT, tag="t1", name="t1")
            nc.vector.stream_shuffle(t1[:], cop[:], PAIRSWAP)
            p1 = stage.tile([128, NTOK], MM_DT, tag="p1", name="p1")
            nc.vector.tensor_mul(p1[:], cop[:], cosm[:])
            p2 = stage.tile([128, NTOK], MM_DT, tag="p2", name="p2")
            if on_act:
                nc.vector.tensor_mul(p2[:], t1[:], sinm[:])
                nc.vector.tensor_add(qk[idx][:], p1[:], p2[:])
            else:
                nc.gpsimd.tensor_mul(p2[:], t1[:], sinm[:])
                nc.gpsimd.tensor_add(qk[idx][:], p1[:], p2[:])

        def v_pair(t0):
            """Project V for token chunks t0, t0+1 into vsb (token-major)."""
            pv = psum.tile([128, NTOK], F32, tag="ps", name="pv")
            for tt in (t0, t0 + 1):
                sl = slice((tt - t0) * 512, (tt - t0 + 1) * 512)
                for kc in range(4):
                    nc.tensor.matmul(
                        pv[:, sl],
                        xT[:, kc, tt * 128:(tt + 1) * 128],
                        wqkv[:, kc, 1024:1536],
                        start=(kc == 0), stop=(kc == 3),
                    )
            for tt in (t0, t0 + 1):
                sl = slice((tt - t0) * 512, (tt - t0 + 1) * 512)
                nc.vector.tensor_copy(
                    vsb[tt][:, :, 0:DH],
                    pv[:, sl].rearrange("p (h d) -> p h d", h=H),
                )

        # ---- prologue: q0, k0, first V chunks ----
        qkv_chunk(0, on_act=True)   # q0
        qkv_chunk(4, on_act=True)   # k0
        v_pair(0)

        def attn_pair(pair, defer):
            """Attention for heads 2*pair, 2*pair+1; defer[j] = work to
            interleave after the PV of j-chunk j."""
            qc, kc = qk[pair], qk[4 + pair]
            po = {s: psum.tile([128, NTOK], F32, tag="po", name=f"po{s}")
                  for s in (0, 1)}
            ets = {}
            for jc in range(8):
                for sub in (0, 1):
                    ps = psum.tile([128, NTOK], F32, tag="ps", name="psw")
                    kap = kc[sub * 64:(sub + 1) * 64, jc * 128:(jc + 1) * 128]
                    for ih in range(2):
                        nc.tensor.matmul(
                            ps[:, ih * 512:(ih + 1) * 512],
                            kap,
                            qc[sub * 64:(sub + 1) * 64, ih * 512:(ih + 1) * 512],
                            start=True, stop=True,
                            tile_position=(sub * 64, 0),
                        )
                    et = epool.tile([128, NTOK], MM_DT, tag="E", name="et")
                    nc.scalar.activation(et[:], ps[:], EXP)
                    ets[(jc, sub)] = et
                for sub in (0, 1):
                    for ih in range(2):
                        nc.tensor.matmul(
                            po[sub][:, ih * 512:(ih + 1) * 512],
                            vsb[jc][:, 2 * pair + sub, :],
                            ets[(jc, sub)][:, ih * 512:(ih + 1) * 512],
                            start=(jc == 0), stop=(jc == 7),
                        )
                if jc in defer:
                    defer[jc]()
            for sub in (0, 1):
                # reciprocal_approx_fast ignores a nonzero input base
                # partition on HW — run it base-0 over all 128 partitions
                # (rows 0..63 are unused garbage) and read rows 64..127.
                rt = rtp.tile([128, NTOK], F32, tag="rt", name="rt")
                nc.vector.reciprocal_approx_fast(rt[:], po[sub][:])
                nc.vector.tensor_mul(
                    obar[pair][sub * 64:(sub + 1) * 64, :],
                    po[sub][0:64, :], rt[64:128, :],
                )

        attn_pair(0, {0: lambda: v_pair(2), 1: lambda: v_pair(4),
                      2: lambda: v_pair(6),
                      3: lambda: qkv_chunk(1, on_act=False),
                      5: lambda: qkv_chunk(5, on_act=False)})
        attn_pair(1, {1: lambda: qkv_chunk(2, on_act=False),
                      4: lambda: qkv_chunk(6, on_act=False)})
        attn_pair(2, {1: lambda: qkv_chunk(3, on_act=False),
                      4: lambda: qkv_chunk(7, on_act=False)})
        attn_pair(3, {})

        # ---- output projection (+bias), fp16 DMA out ----
        for oc in range(4):
            pf = psum.tile([128, NTOK], F32, tag="ps", name="pf")
            for fc in range(4):
                for ih in range(2):
                    nc.tensor.matmul(
                        pf[:, ih * 512:(ih + 1) * 512],
                        wout[:, fc, oc * 128:(oc + 1) * 128],
                        obar[fc][:, ih * 512:(ih + 1) * 512],
                        start=(fc == 0), stop=(fc == 3),
                    )
            nc.vector.tensor_scalar_add(outsb[oc][:], pf[:], bout[:, oc:oc + 1])
            nc.sync.dma_start(out_d[oc * 128:(oc + 1) * 128, :], outsb[oc][:])

    nc.compile()
    return nc


def host_prep(x, W_qkv, W_out, b_out, sin, cos):
    """Build the per-core input tensors (host-side prep, incl. x transpose)."""
    x = np.asarray(x, dtype=np.float32)
    W_qkv = np.asarray(W_qkv, dtype=np.float32).copy()
    W_out = np.ascontiguousarray(np.asarray(W_out, dtype=np.float32))
    b_out = np.ascontiguousarray(np.asarray(b_out, dtype=np.float32))
    sin = np.asarray(sin, dtype=np.float32)
    cos = np.asarray(cos, dtype=np.float32)

    # fold q scaling into W_qkv's q block
    W_qkv[:, 0:H * DH] *= SCALE

    # masked, feature-major cos/sin tiles [128, 1024]
    dloc = np.arange(128) % DH
    sign = np.where(np.arange(128) % 2 == 0, -1.0, 1.0).astype(np.float32)
    cosT = cos.T.astype(np.float32)  # [32, 1024]
    sinT = sin.T.astype(np.float32)
    cosm = np.ones((128, NTOK), dtype=np.float32)
    sinm = np.zeros((128, NTOK), dtype=np.float32)
    rot_rows = dloc < ROT
    cosm[rot_rows] = cosT[dloc[rot_rows]]
    sinm[rot_rows] = sinT[dloc[rot_rows]] * sign[rot_rows][:, None]

    shared = {
        "wqkv": W_qkv.astype(np.float16), "wout": W_out.astype(np.float16),
        "bout": b_out, "cosm": cosm.astype(np.float16),
        "sinm": sinm.astype(np.float16),
    }
    in_maps = []
    for c in range(NCORES):
        bi, fi = c // NF, c % NF
        m = dict(shared)
        m["xt"] = np.ascontiguousarray(x[bi, fi * NTOK:(fi + 1) * NTOK, :].T).astype(np.float16)
        in_maps.append(m)
    return in_maps


_CACHED_NC = None


def kernel(x, W_qkv, W_out, b_out, sin, cos, f=4, **run_kwargs):
    global _CACHED_NC
    assert int(f) == NF
    in_maps = host_prep(x, W_qkv, W_out, b_out, sin, cos)
    if _CACHED_NC is None:
        _CACHED_NC = build_program()
    res = run_bass_kernel_spmd(
        _CACHED_NC, in_maps, core_ids=list(range(NCORES)), **run_kwargs
    )
    out = np.empty((B, N, DIM), dtype=np.float32)
    for c in range(NCORES):
        bi, fi = c // NF, c % NF
        out[bi, fi * NTOK:(fi + 1) * NTOK, :] = res.results[c]["out_t"].T.astype(np.float32)
    if run_kwargs:
        kernel.last_results = res
    return out


# revision 6
# speedup vs baseline: 1.1214x; 1.1214x over previous
"""Trainium2 Bass kernel for per-frame multi-head attention with partial RoPE.

Problem (hardcoded): b=2, N=4096, dim=512, H=8, DH=64, f=4 frames of n=1024
tokens, ROT_DIM=32 partial rotary, softmax attention per (b, h, frame) block,
then output projection.

Sharding: 8 cores = (batch, frame) pairs. Each core runs all 8 heads for one
1024-token frame — fully independent, no collectives.

v2 design notes (per core):
  - The ACT (scalar) engine is the hard floor: 64 exps of [128,1024] at
    ~1.04us each (~66us). It does ONLY exp; the table set is preloaded by a
    junk exp at t=0. All PSUM->SBUF evacuation rides DVE (plus two ACT copies
    in the prologue while ACT is otherwise idle).
  - PSUM (8 banks): tag "ps" 2x[128,1024] is the S^T ring (also carries
    deferred QKV chunks, V chunks, and the output projection); tag "po"
    2x[128,1024] holds the two PV accumulators of the current head pair.
  - Softmax denominator for free: the PV stationary is widened to 128 cols
    where cols 64..127 are all-ones, so po rows 64..127 hold l[i] broadcast
    across 64 partitions. Normalize = reciprocal_approx_fast + one mul.
  - S^T runs as concurrent PE row-tiles (two heads at tile_position (0,0) /
    (64,0)); q-scale is folded into W_qkv on the host; exp is applied without
    max-subtraction (logits ~N(0,1)).
  - RoPE uses host-precomputed masked cos/sin tiles; deferred chunks push the
    second mul + add to the (otherwise idle) GpSimd engine.
  - ~30 junk matmuls at t=0 warm the PE HAM clock gate during the DMA fill;
    DMA order puts xT + the first q/k weight stripes first.
  - Output is written as fp16 [512,1024]; the host transposes/casts.
"""

from contextlib import ExitStack

import numpy as np

import concourse.bass as bass
import concourse.tile as tile
from concourse import bacc
from concourse import mybir
from concourse.bass_utils import run_bass_kernel_spmd

F32 = mybir.dt.float32
F16 = mybir.dt.float16
BF16 = mybir.dt.bfloat16

B, N, DIM = 2, 4096, 512
H, DH = 8, 64
NF = 4                # frames
NTOK = 1024           # tokens per frame
ROT = 32
SCALE = DH ** -0.5
NCORES = 8

PAIRSWAP = [i ^ 1 for i in range(32)]
MM_DT = F16
N_WARM = 30


def build_program():
    """Build the single-core Bass/Tile program (SPMD across 8 cores)."""
    nc = bacc.Bacc(trn_type="TRN2", target_bir_lowering=False, debug=False)

    xt_d = nc.dram_tensor("xt", [DIM, NTOK], MM_DT, kind="ExternalInput").ap()
    wqkv_d = nc.dram_tensor("wqkv", [DIM, 3 * H * DH], MM_DT, kind="ExternalInput").ap()
    wout_d = nc.dram_tensor("wout", [H * DH, DIM], MM_DT, kind="ExternalInput").ap()
    bout_d = nc.dram_tensor("bout", [DIM], F32, kind="ExternalInput").ap()
    cosm_d = nc.dram_tensor("cosm", [128, NTOK], MM_DT, kind="ExternalInput").ap()
    sinm_d = nc.dram_tensor("sinm", [128, NTOK], MM_DT, kind="ExternalInput").ap()
    out_d = nc.dram_tensor("out_t", [DIM, NTOK], MM_DT, kind="ExternalOutput").ap()

    EXP = mybir.ActivationFunctionType.Exp

    with tile.TileContext(nc) as tc, ExitStack() as ctx:
        const = ctx.enter_context(tc.tile_pool(name="const", bufs=1))
        big = ctx.enter_context(tc.tile_pool(name="big", bufs=1))
        stage = ctx.enter_context(tc.tile_pool(name="stage", bufs=2))
        epool = ctx.enter_context(tc.tile_pool(name="E", bufs=8))
        rtp = ctx.enter_context(tc.tile_pool(name="rtp", bufs=2))
        psum = ctx.enter_context(tc.tile_pool(name="ps", bufs=2, space="PSUM"))

        # ---- SBUF tiles ----
        wqkv = const.tile([128, 4, 3 * H * DH], MM_DT, tag="wqkv", name="wqkv_sb")
        xT = const.tile([128, 4, NTOK], MM_DT, tag="xT", name="xT")
        wout = const.tile([128, 4, DIM], MM_DT, tag="wout", name="wout_sb")
        bout = const.tile([128, 4], F32, tag="bout", name="bout_sb")
        cosm = const.tile([128, NTOK], MM_DT, tag="cosm", name="cosm_sb")
        sinm = const.tile([128, NTOK], MM_DT, tag="sinm", name="sinm_sb")
        junkW = const.tile([128, 128], MM_DT, tag="junkW", name="junkW")
        junkX = const.tile([128, 128], MM_DT, tag="junkX", name="junkX")
        junkE = const.tile([128, 16], MM_DT, tag="junkE", name="junkE")

        # q chunks 0..3 then k chunks 0..3 (roped, fp16, feature-major)
        qk = [big.tile([128, NTOK], MM_DT, tag=f"qk{i}", name=f"qk{i}")
              for i in range(8)]
        # V token-major per j-chunk: [128 tok, head, DH cols | 64 ones cols]
        vsb = [big.tile([128, H, 128], MM_DT, tag=f"v{t}", name=f"v{t}")
               for t in range(8)]
        obar = [big.tile([128, NTOK], MM_DT, tag=f"ob{c}", name=f"ob{c}")
                for c in range(4)]
        outsb = [big.tile([128, NTOK], MM_DT, tag=f"os{c}", name=f"os{c}")
                 for c in range(4)]

        # ---- warmup: ACT table load + PE HAM while DMAs fill ----
        nc.vector.memset(junkW[:], 0.03125)
        nc.vector.memset(junkX[:], 0.03125)
        nc.scalar.activation(junkE[:], junkX[:, 0:16], EXP)
        for i in range(N_WARM):
            wps = psum.tile([128, 512], F32, tag="po", name="warm")
            nc.tensor.matmul(wps[:, 0:128], junkW[:], junkX[:],
                             start=True, stop=True)
        # ones columns of the PV stationaries (cols DH..127)
        for t in range(8):
            nc.vector.memset(vsb[t][:, :, DH:128], 1.0)

        # ---- DMAs, critical-path order ----
        xt_r = xt_d.rearrange("(kc p) t -> p kc t", p=128)
        wqkv_r = wqkv_d.rearrange("(kc p) c -> p kc c", p=128)
        for tq in range(4):
            nc.sync.dma_start(xT[:, :, tq * 256:(tq + 1) * 256],
                              xt_r[:, :, tq * 256:(tq + 1) * 256])
        nc.sync.dma_start(wqkv[:, :, 0:128], wqkv_r[:, :, 0:128])          # q0
        nc.sync.dma_start(wqkv[:, :, 512:640], wqkv_r[:, :, 512:640])      # k0
        nc.sync.dma_start(wqkv[:, :, 1024:1536], wqkv_r[:, :, 1024:1536])  # v
        nc.sync.dma_start(cosm[:], cosm_d)
        nc.sync.dma_start(sinm[:], sinm_d)
        nc.sync.dma_start(wqkv[:, :, 128:512], wqkv_r[:, :, 128:512])      # q1-3
        nc.sync.dma_start(wqkv[:, :, 640:1024], wqkv_r[:, :, 640:1024])    # k1-3
        nc.sync.dma_start(wout[:], wout_d.rearrange("(kc p) c -> p kc c", p=128))
        nc.sync.dma_start(bout[:], bout_d.rearrange("(c p) -> p c", p=128))

        def qkv_chunk(idx, on_act):
            """Project one q/k chunk (2 heads) and rope it into qk[idx]."""
            col0 = idx * 128 if idx < 4 else 512 + (idx - 4) * 128
            pq = psum.tile([128, NTOK], F32, tag="ps", name="pq")
            for kc in range(4):
                for ih in range(2):
                    nc.tensor.matmul(
                        pq[:, ih * 512:(ih + 1) * 512],
                        wqkv[:, kc, col0:col0 + 128],
                        xT[:, kc, ih * 512:(ih + 1) * 512],
                        start=(kc == 0), stop=(kc == 3),
                    )
            cop = stage.tile([128, NTOK], MM_DT, tag="cop", name="cop")
            if on_act:
                nc.scalar.copy(cop[:], pq[:])
            else:
                nc.vector.tensor_copy(cop[:], pq[:])
            t1 = stage.tile([128, NTOK], MM_DT, tag="t1", name="t1")
            nc.vector.stream_shuffle(t1[:], cop[:], PAIRSWAP)
            p1 = stage.tile([128, NTOK], MM_DT, tag="p1", name="p1")
            nc.vector.tensor_mul(p1[:], cop[:], cosm[:])
            p2 = stage.tile([128, NTOK], MM_DT, tag="p2", name="p2")
            nc.vector.tensor_mul(p2[:], t1[:], sinm[:])
            nc.vector.tensor_add(qk[idx][:], p1[:], p2[:])

        def v_pair(t0):
            """Project V for token chunks t0, t0+1 into vsb (token-major)."""
            pv = psum.tile([128, NTOK], F32, tag="ps", name="pv")
            for tt in (t0, t0 + 1):
                sl = slice((tt - t0) * 512, (tt - t0 + 1) * 512)
                for kc in range(4):
                    nc.tensor.matmul(
                        pv[:, sl],
                        xT[:, kc, tt * 128:(tt + 1) * 128],
                        wqkv[:, kc, 1024:1536],
                        start=(kc == 0), stop=(kc == 3),
                    )
            for tt in (t0, t0 + 1):
                sl = slice((tt - t0) * 512, (tt - t0 + 1) * 512)
                nc.vector.tensor_copy(
                    vsb[tt][:, :, 0:DH],
                    pv[:, sl].rearrange("p (h d) -> p h d", h=H),
                )

        # partial output projection over feature chunks fc0..fc1 into SBUF
        pacc = [big.tile([128, NTOK], MM_DT, tag=f"pa{c}", name=f"pa{c}")
                for c in range(4)]

        def out_partial(oc):
            """pacc[oc] = sum_{fc<3} wout[fc]^T obar[fc]  (runs inside pair 3)."""
            pf = psum.tile([128, NTOK], F32, tag="ps", name="pfa")
            for fc in range(3):
                for ih in range(2):
                    nc.tensor.matmul(
                        pf[:, ih * 512:(ih + 1) * 512],
                        wout[:, fc, oc * 128:(oc + 1) * 128],
                        obar[fc][:, ih * 512:(ih + 1) * 512],
                        start=(fc == 0), stop=(fc == 2),
                    )
            nc.vector.tensor_copy(pacc[oc][:], pf[:])

        # ---- prologue: q0, k0, q1, k1, V chunks 0-3 ----
        qkv_chunk(0, on_act=True)   # q0
        qkv_chunk(4, on_act=True)   # k0
        v_pair(0)
        v_pair(2)
        qkv_chunk(1, on_act=True)   # q1
        qkv_chunk(5, on_act=True)   # k1

        def attn_pair(pair, defer):
            """Attention for heads 2*pair, 2*pair+1. PV lags S^T by 2 j-chunks
            so the PE never sits in FIFO behind an exp it doesn't need.
            defer[j] = extra work issued after step j's S^T."""
            qc, kc = qk[pair], qk[4 + pair]
            po = {s: psum.tile([128, NTOK], F32, tag="po", name=f"po{s}")
                  for s in (0, 1)}
            ets = {}

            def pv(jc):
                for sub in (0, 1):
                    for ih in range(2):
                        nc.tensor.matmul(
                            po[sub][:, ih * 512:(ih + 1) * 512],
                            vsb[jc][:, 2 * pair + sub, :],
                            ets[(jc, sub)][:, ih * 512:(ih + 1) * 512],
                            start=(jc == 0), stop=(jc == 7),
                        )

            for jc in range(8):
                for sub in (0, 1):
                    ps = psum.tile([128, NTOK], F32, tag="ps", name="psw")
                    kap = kc[sub * 64:(sub + 1) * 64, jc * 128:(jc + 1) * 128]
                    for ih in range(2):
                        nc.tensor.matmul(
                            ps[:, ih * 512:(ih + 1) * 512],
                            kap,
                            qc[sub * 64:(sub + 1) * 64, ih * 512:(ih + 1) * 512],
                            start=True, stop=True,
                            tile_position=(sub * 64, 0),
                        )
                    et = epool.tile([128, NTOK], MM_DT, tag="E", name="et")
                    nc.scalar.activation(et[:], ps[:], EXP)
                    ets[(jc, sub)] = et
                if jc >= 2:
                    pv(jc - 2)
                if jc in defer:
                    defer[jc]()
            pv(6)
            pv(7)
            for sub in (0, 1):
                # reciprocal_approx_fast ignores a nonzero input base
                # partition on HW — run it base-0 over all 128 partitions
                # (rows 0..63 are unused garbage) and read rows 64..127.
                rt = rtp.tile([128, NTOK], F32, tag="rt", name="rt")
                nc.vector.reciprocal_approx_fast(rt[:], po[sub][:])
                nc.vector.tensor_mul(
                    obar[pair][sub * 64:(sub + 1) * 64, :],
                    po[sub][0:64, :], rt[64:128, :],
                )

        attn_pair(0, {2: lambda: v_pair(4), 4: lambda: v_pair(6)})
        attn_pair(1, {2: lambda: qkv_chunk(2, on_act=False),
                      5: lambda: qkv_chunk(6, on_act=False)})
        attn_pair(2, {2: lambda: qkv_chunk(3, on_act=False),
                      5: lambda: qkv_chunk(7, on_act=False)})
        attn_pair(3, {3: lambda: out_partial(0), 4: lambda: out_partial(1),
                      5: lambda: out_partial(2), 6: lambda: out_partial(3)})

        # ---- output projection tail: last feature chunk + partial + bias ----
        ADD = mybir.AluOpType.add
        for oc in range(4):
            pf = psum.tile([128, NTOK], F32, tag="ps", name="pf")
            for ih in range(2):
                nc.tensor.matmul(
                    pf[:, ih * 512:(ih + 1) * 512],
                    wout[:, 3, oc * 128:(oc + 1) * 128],
                    obar[3][:, ih * 512:(ih + 1) * 512],
                    start=True, stop=True,
                )
            # outsb = (pf + bias) + pacc in one DVE pass
            nc.vector.scalar_tensor_tensor(
                outsb[oc][:], pf[:], bout[:, oc:oc + 1], pacc[oc][:],
                ADD, ADD,
            )
            nc.sync.dma_start(out_d[oc * 128:(oc + 1) * 128, :], outsb[oc][:])

    nc.compile()
    return nc


def host_prep(x, W_qkv, W_out, b_out, sin, cos):
    """Build the per-core input tensors (host-side prep, incl. x transpose)."""
    x = np.asarray(x, dtype=np.float32)
    W_qkv = np.asarray(W_qkv, dtype=np.float32).copy()
    W_out = np.ascontiguousarray(np.asarray(W_out, dtype=np.float32))
    b_out = np.ascontiguousarray(np.asarray(b_out, dtype=np.float32))
    sin = np.asarray(sin, dtype=np.float32)
    cos = np.asarray(cos, dtype=np.float32)

    # fold q scaling into W_qkv's q block
    W_qkv[:, 0:H * DH] *= SCALE

    # masked, feature-major cos/sin tiles [128, 1024]
    dloc = np.arange(128) % DH
    sign = np.where(np.arange(128) % 2 == 0, -1.0, 1.0).astype(np.float32)
    cosT = cos.T.astype(np.float32)  # [32, 1024]
    sinT = sin.T.astype(np.float32)
    cosm = np.ones((128, NTOK), dtype=np.float32)
    sinm = np.zeros((128, NTOK), dtype=np.float32)
    rot_rows = dloc < ROT
    cosm[rot_rows] = cosT[dloc[rot_rows]]
    sinm[rot_rows] = sinT[dloc[rot_rows]] * sign[rot_rows][:, None]

    shared = {
        "wqkv": W_qkv.astype(np.float16), "wout": W_out.astype(np.float16),
        "bout": b_out, "cosm": cosm.astype(np.float16),
        "sinm": sinm.astype(np.float16),
    }
    in_maps = []
    for c in range(NCORES):
        bi, fi = c // NF, c % NF
        m = dict(shared)
        m["xt"] = np.ascontiguousarray(x[bi, fi * NTOK:(fi + 1) * NTOK, :].T).astype(np.float16)
        in_maps.append(m)
    return in_maps


_CACHED_NC = None


def kernel(x, W_qkv, W_out, b_out, sin, cos, f=4, **run_kwargs):
    global _CACHED_NC
    assert int(f) == NF
    in_maps = host_prep(x, W_qkv, W_out, b_out, sin, cos)
    if _CACHED_NC is None:
        _CACHED_NC = build_program()
    res = run_bass_kernel_spmd(
        _CACHED_NC, in_maps, core_ids=list(range(NCORES)), **run_kwargs
    )
    out = np.empty((B, N, DIM), dtype=np.float32)
    for c in range(NCORES):
        bi, fi = c // NF, c % NF
        out[bi, fi * NTOK:(fi + 1) * NTOK, :] = res.results[c]["out_t"].T.astype(np.float32)
    if run_kwargs:
        kernel.last_results = res
    return out


# revision 12
# speedup vs baseline: 1.3390x; 1.1941x over previous
"""Trainium2 Bass kernel for per-frame multi-head attention with partial RoPE.

Problem (hardcoded): b=2, N=4096, dim=512, H=8, DH=64, f=4 frames of n=1024
tokens, ROT_DIM=32 partial rotary, softmax attention per (b, h, frame) block,
then output projection.

Sharding: 8 cores = (batch, frame) pairs. Each core runs all 8 heads for one
1024-token frame — fully independent, no collectives.

v2 design notes (per core):
  - The ACT (scalar) engine is the hard floor: 64 exps of [128,1024] at
    ~1.04us each (~66us). It does ONLY exp; the table set is preloaded by a
    junk exp at t=0. All PSUM->SBUF evacuation rides DVE (plus two ACT copies
    in the prologue while ACT is otherwise idle).
  - PSUM (8 banks): tag "ps" 2x[128,1024] is the S^T ring (also carries
    deferred QKV chunks, V chunks, and the output projection); tag "po"
    2x[128,1024] holds the two PV accumulators of the current head pair.
  - Softmax denominator for free: the PV stationary is widened to 128 cols
    where cols 64..127 are all-ones, so po rows 64..127 hold l[i] broadcast
    across 64 partitions. Normalize = reciprocal_approx_fast + one mul.
  - S^T runs as concurrent PE row-tiles (two heads at tile_position (0,0) /
    (64,0)); q-scale is folded into W_qkv on the host; exp is applied without
    max-subtraction (logits ~N(0,1)).
  - RoPE uses host-precomputed masked cos/sin tiles; deferred chunks push the
    second mul + add to the (otherwise idle) GpSimd engine.
  - ~30 junk matmuls at t=0 warm the PE HAM clock gate during the DMA fill;
    DMA order puts xT + the first q/k weight stripes first.
  - Output is written as fp16 [512,1024]; the host transposes/casts.
"""

from contextlib import ExitStack

import numpy as np

import concourse.bass as bass
import concourse.tile as tile
from concourse import bacc
from concourse import mybir
from concourse.bass_utils import run_bass_kernel_spmd

F32 = mybir.dt.float32
F16 = mybir.dt.float16
BF16 = mybir.dt.bfloat16

B, N, DIM = 2, 4096, 512
H, DH = 8, 64
NF = 4                # frames
NTOK = 1024           # tokens per frame
ROT = 32
SCALE = DH ** -0.5
NCORES = 8

PAIRSWAP = [i ^ 1 for i in range(32)]
MM_DT = F16
N_WARM = 0


def build_program():
    """Build the single-core Bass/Tile program (SPMD across 8 cores)."""
    nc = bacc.Bacc(trn_type="TRN2", target_bir_lowering=False, debug=False)

    xt_d = nc.dram_tensor("xt", [DIM, NTOK], MM_DT, kind="ExternalInput").ap()
    wqkv_d = nc.dram_tensor("wqkv", [DIM, 3 * H * DH], MM_DT, kind="ExternalInput").ap()
    wout_d = nc.dram_tensor("wout", [H * DH, DIM], MM_DT, kind="ExternalInput").ap()
    bout_d = nc.dram_tensor("bout", [DIM], F32, kind="ExternalInput").ap()
    cosm_d = nc.dram_tensor("cosm", [128, NTOK], MM_DT, kind="ExternalInput").ap()
    sinm_d = nc.dram_tensor("sinm", [128, NTOK], MM_DT, kind="ExternalInput").ap()
    out_d = nc.dram_tensor("out_t", [DIM, NTOK], MM_DT, kind="ExternalOutput").ap()

    EXP = mybir.ActivationFunctionType.Exp

    with tile.TileContext(nc) as tc, ExitStack() as ctx:
        const = ctx.enter_context(tc.tile_pool(name="const", bufs=1))
        big = ctx.enter_context(tc.tile_pool(name="big", bufs=1))
        stage = ctx.enter_context(tc.tile_pool(name="stage", bufs=2))
        epool = ctx.enter_context(tc.tile_pool(name="E", bufs=10))
        rtp = ctx.enter_context(tc.tile_pool(name="rtp", bufs=2))
        psum = ctx.enter_context(tc.tile_pool(name="ps", bufs=2, space="PSUM"))

        # ---- SBUF tiles ----
        wqkv = const.tile([128, 4, 3 * H * DH], MM_DT, tag="wqkv", name="wqkv_sb")
        xT = const.tile([128, 4, NTOK], MM_DT, tag="xT", name="xT")
        wout = const.tile([128, 4, DIM], MM_DT, tag="wout", name="wout_sb")
        bout = const.tile([128, 4], F32, tag="bout", name="bout_sb")
        cosm = const.tile([128, NTOK], MM_DT, tag="cosm", name="cosm_sb")
        sinm = const.tile([128, NTOK], MM_DT, tag="sinm", name="sinm_sb")
        junkW = const.tile([128, 128], MM_DT, tag="junkW", name="junkW")
        junkX = const.tile([128, 128], MM_DT, tag="junkX", name="junkX")
        junkE = const.tile([128, 16], MM_DT, tag="junkE", name="junkE")

        # q chunks 0..3 then k chunks 0..3 (roped, fp16, feature-major)
        qk = [big.tile([128, NTOK], MM_DT, tag=f"qk{i}", name=f"qk{i}")
              for i in range(8)]
        # V token-major per j-chunk: [128 tok, head, DH cols | 64 ones cols]
        vsb = [big.tile([128, H, 128], MM_DT, tag=f"v{t}", name=f"v{t}")
               for t in range(8)]
        obar = [big.tile([128, NTOK], MM_DT, tag=f"ob{c}", name=f"ob{c}")
                for c in range(4)]
        outsb = [big.tile([128, NTOK], MM_DT, tag=f"os{c}", name=f"os{c}")
                 for c in range(4)]

        # ---- warmup: ACT table load + PE HAM while DMAs fill ----
        nc.vector.memset(junkW[:], 0.03125)
        nc.vector.memset(junkX[:], 0.03125)
        nc.scalar.activation(junkE[:], junkX[:, 0:16], EXP)
        for i in range(N_WARM):
            wps = psum.tile([128, 512], F32, tag="po", name="warm")
            nc.tensor.matmul(wps[:, 0:128], junkW[:], junkX[:],
                             start=True, stop=True)
        # ones columns of the PV stationaries (cols DH..127)
        for t in range(8):
            nc.vector.memset(vsb[t][:, :, DH:128], 1.0)

        # ---- DMAs, critical-path order ----
        xt_r = xt_d.rearrange("(kc p) t -> p kc t", p=128)
        wqkv_r = wqkv_d.rearrange("(kc p) c -> p kc c", p=128)
        for tq in range(4):
            nc.sync.dma_start(xT[:, :, tq * 256:(tq + 1) * 256],
                              xt_r[:, :, tq * 256:(tq + 1) * 256])
        nc.sync.dma_start(wqkv[:, :, 0:128], wqkv_r[:, :, 0:128])          # q0
        nc.sync.dma_start(wqkv[:, :, 512:640], wqkv_r[:, :, 512:640])      # k0
        nc.sync.dma_start(wqkv[:, :, 1024:1536], wqkv_r[:, :, 1024:1536])  # v
        nc.sync.dma_start(cosm[:], cosm_d)
        nc.sync.dma_start(sinm[:], sinm_d)
        nc.sync.dma_start(wqkv[:, :, 128:512], wqkv_r[:, :, 128:512])      # q1-3
        nc.sync.dma_start(wqkv[:, :, 640:1024], wqkv_r[:, :, 640:1024])    # k1-3
        nc.sync.dma_start(wout[:], wout_d.rearrange("(kc p) c -> p kc c", p=128))
        nc.sync.dma_start(bout[:], bout_d.rearrange("(c p) -> p c", p=128))

        def qkv_chunk(idx, on_act):
            """Project one q/k chunk (2 heads) and rope it into qk[idx]."""
            col0 = idx * 128 if idx < 4 else 512 + (idx - 4) * 128
            pq = psum.tile([128, NTOK], F32, tag="ps", name="pq")
            for kc in range(4):
                for ih in range(2):
                    nc.tensor.matmul(
                        pq[:, ih * 512:(ih + 1) * 512],
                        wqkv[:, kc, col0:col0 + 128],
                        xT[:, kc, ih * 512:(ih + 1) * 512],
                        start=(kc == 0), stop=(kc == 3),
                    )
            cop = stage.tile([128, NTOK], MM_DT, tag="cop", name="cop")
            if on_act:
                nc.scalar.copy(cop[:], pq[:])
            else:
                nc.vector.tensor_copy(cop[:], pq[:])
            t1 = stage.tile([128, NTOK], MM_DT, tag="t1", name="t1")
            nc.vector.stream_shuffle(t1[:], cop[:], PAIRSWAP)
            p1 = stage.tile([128, NTOK], MM_DT, tag="p1", name="p1")
            nc.vector.tensor_mul(p1[:], cop[:], cosm[:])
            p2 = stage.tile([128, NTOK], MM_DT, tag="p2", name="p2")
            nc.vector.tensor_mul(p2[:], t1[:], sinm[:])
            nc.vector.tensor_add(qk[idx][:], p1[:], p2[:])

        def v_pair(t0):
            """Project V for token chunks t0, t0+1 into vsb (token-major)."""
            pv = psum.tile([128, NTOK], F32, tag="ps", name="pv")
            for tt in (t0, t0 + 1):
                sl = slice((tt - t0) * 512, (tt - t0 + 1) * 512)
                for kc in range(4):
                    nc.tensor.matmul(
                        pv[:, sl],
                        xT[:, kc, tt * 128:(tt + 1) * 128],
                        wqkv[:, kc, 1024:1536],
                        start=(kc == 0), stop=(kc == 3),
                    )
            for tt in (t0, t0 + 1):
                sl = slice((tt - t0) * 512, (tt - t0 + 1) * 512)
                nc.vector.tensor_copy(
                    vsb[tt][:, :, 0:DH],
                    pv[:, sl].rearrange("p (h d) -> p h d", h=H),
                )

        # partial output projection over feature chunks fc0..fc1 into SBUF
        pacc = [big.tile([128, NTOK], MM_DT, tag=f"pa{c}", name=f"pa{c}")
                for c in range(4)]

        def out_partial(oc):
            """pacc[oc] = bias + sum_{fc<3} wout[fc]^T obar[fc] (inside pair 3)."""
            pf = psum.tile([128, NTOK], F32, tag="ps", name="pfa")
            for fc in range(3):
                for ih in range(2):
                    nc.tensor.matmul(
                        pf[:, ih * 512:(ih + 1) * 512],
                        wout[:, fc, oc * 128:(oc + 1) * 128],
                        obar[fc][:, ih * 512:(ih + 1) * 512],
                        start=(fc == 0), stop=(fc == 2),
                    )
            nc.vector.tensor_scalar_add(pacc[oc][:], pf[:], bout[:, oc:oc + 1])

        # ---- prologue: q0, k0, V chunks 0-3 (fills PE while DVE ropes) ----
        qkv_chunk(0, on_act=True)   # q0
        qkv_chunk(4, on_act=True)   # k0
        v_pair(0)
        v_pair(2)

        def attn_pair(pair, defer):
            """Attention for heads 2*pair, 2*pair+1. PV lags S^T by 2 j-chunks
            so the PE never sits in FIFO behind an exp it doesn't need.
            defer[j] = extra work issued after step j's S^T. Returns a
            finish() that issues the last two PV steps + normalize — the
            caller threads it into the NEXT pair's defers so the pair
            boundary overlaps the next pair's S^T stream."""
            qc, kc = qk[pair], qk[4 + pair]
            po = {s: psum.tile([128, NTOK], F32, tag="po", name=f"po{s}")
                  for s in (0, 1)}
            ets = {}

            def pv(jc):
                for sub in (0, 1):
                    for ih in range(2):
                        nc.tensor.matmul(
                            po[sub][:, ih * 512:(ih + 1) * 512],
                            vsb[jc][:, 2 * pair + sub, :],
                            ets[(jc, sub)][:, ih * 512:(ih + 1) * 512],
                            start=(jc == 0), stop=(jc == 7),
                        )

            for jc in range(8):
                for sub in (0, 1):
                    ps = psum.tile([128, NTOK], F32, tag="ps", name="psw")
                    kap = kc[sub * 64:(sub + 1) * 64, jc * 128:(jc + 1) * 128]
                    for ih in range(2):
                        nc.tensor.matmul(
                            ps[:, ih * 512:(ih + 1) * 512],
                            kap,
                            qc[sub * 64:(sub + 1) * 64, ih * 512:(ih + 1) * 512],
                            start=True, stop=True,
                            tile_position=(sub * 64, 0),
                        )
                    et = epool.tile([128, NTOK], MM_DT, tag="E", name="et")
                    nc.scalar.activation(et[:], ps[:], EXP)
                    ets[(jc, sub)] = et
                if jc >= 2:
                    pv(jc - 2)
                if jc in defer:
                    defer[jc]()

            def finish():
                pv(6)
                pv(7)
                for sub in (0, 1):
                    # reciprocal_approx_fast ignores a nonzero input base
                    # partition on HW — run it base-0 over all 128 partitions
                    # (rows 0..63 are unused garbage) and read rows 64..127.
                    rt = rtp.tile([128, NTOK], F32, tag="rt", name="rt")
                    nc.vector.reciprocal_approx_fast(rt[:], po[sub][:])
                    nc.vector.tensor_mul(
                        obar[pair][sub * 64:(sub + 1) * 64, :],
                        po[sub][0:64, :], rt[64:128, :],
                    )
            return finish

        f0 = attn_pair(0, {0: lambda: qkv_chunk(1, on_act=False),
                           3: lambda: qkv_chunk(5, on_act=False),
                           5: lambda: v_pair(4), 6: lambda: v_pair(6)})
        f1 = attn_pair(1, {0: lambda: f0(),
                           2: lambda: qkv_chunk(2, on_act=False),
                           5: lambda: qkv_chunk(6, on_act=False)})
        f2 = attn_pair(2, {0: lambda: f1(),
                           2: lambda: qkv_chunk(3, on_act=False),
                           5: lambda: qkv_chunk(7, on_act=False)})
        f3 = attn_pair(3, {0: lambda: f2(),
                           3: lambda: out_partial(0), 4: lambda: out_partial(1),
                           5: lambda: out_partial(2), 6: lambda: out_partial(3)})
        f3()

        # ---- output projection tail: last feature chunk + partial ----
        for oc in range(4):
            pf = psum.tile([128, NTOK], F32, tag="ps", name="pf")
            for ih in range(2):
                nc.tensor.matmul(
                    pf[:, ih * 512:(ih + 1) * 512],
                    wout[:, 3, oc * 128:(oc + 1) * 128],
                    obar[3][:, ih * 512:(ih + 1) * 512],
                    start=True, stop=True,
                )
            nc.vector.tensor_add(outsb[oc][:], pf[:], pacc[oc][:])
            nc.sync.dma_start(out_d[oc * 128:(oc + 1) * 128, :], outsb[oc][:])

    nc.compile()
    return nc


def host_prep(x, W_qkv, W_out, b_out, sin, cos):
    """Build the per-core input tensors (host-side prep, incl. x transpose)."""
    x = np.asarray(x, dtype=np.float32)
    W_qkv = np.asarray(W_qkv, dtype=np.float32).copy()
    W_out = np.ascontiguousarray(np.asarray(W_out, dtype=np.float32))
    b_out = np.ascontiguousarray(np.asarray(b_out, dtype=np.float32))
    sin = np.asarray(sin, dtype=np.float32)
    cos = np.asarray(cos, dtype=np.float32)

    # fold q scaling into W_qkv's q block
    W_qkv[:, 0:H * DH] *= SCALE

    # masked, feature-major cos/sin tiles [128, 1024]
    dloc = np.arange(128) % DH
    sign = np.where(np.arange(128) % 2 == 0, -1.0, 1.0).astype(np.float32)
    cosT = cos.T.astype(np.float32)  # [32, 1024]
    sinT = sin.T.astype(np.float32)
    cosm = np.ones((128, NTOK), dtype=np.float32)
    sinm = np.zeros((128, NTOK), dtype=np.float32)
    rot_rows = dloc < ROT
    cosm[rot_rows] = cosT[dloc[rot_rows]]
    sinm[rot_rows] = sinT[dloc[rot_rows]] * sign[rot_rows][:, None]

    shared = {
        "wqkv": W_qkv.astype(np.float16), "wout": W_out.astype(np.float16),
        "bout": b_out, "cosm": cosm.astype(np.float16),
        "sinm": sinm.astype(np.float16),
    }
    in_maps = []
    for c in range(NCORES):
        bi, fi = c // NF, c % NF
        m = dict(shared)
        m["xt"] = np.ascontiguousarray(x[bi, fi * NTOK:(fi + 1) * NTOK, :].T).astype(np.float16)
        in_maps.append(m)
    return in_maps


_CACHED_NC = None


def kernel(x, W_qkv, W_out, b_out, sin, cos, f=4, **run_kwargs):
    global _CACHED_NC
    assert int(f) == NF
    in_maps = host_prep(x, W_qkv, W_out, b_out, sin, cos)
    if _CACHED_NC is None:
        _CACHED_NC = build_program()
    res = run_bass_kernel_spmd(
        _CACHED_NC, in_maps, core_ids=list(range(NCORES)), **run_kwargs
    )
    out = np.empty((B, N, DIM), dtype=np.float32)
    for c in range(NCORES):
        bi, fi = c // NF, c % NF
        out[bi, fi * NTOK:(fi + 1) * NTOK, :] = res.results[c]["out_t"].T.astype(np.float32)
    if run_kwargs:
        kernel.last_results = res
    return out
